# revision 7
# baseline (speedup 1.0000x reference)
"""Trainium2 Bass kernel for nn_CrossAttention (B=8, L=K=512, M=N=P=D=64).

One batch per NeuronCore (8 cores, data-parallel, no collectives).

Math per batch b:
  scoresT[k,l] = scale * (K @ Q^T)                  # PE, contract D=64
  ET = exp(scale*scoresT)                           # ACT (no max-sub: |s|<~45)
  vkc[k,n] = sum_p vk[k,p,n]*vexp[k,p]              # DVE+GPSIMD mult, DVE bf16 tree
  [tmp|sums][l,(n|1)] = ET_chunk^T @ [vkc|1]        # PE accumulate, l on partitions
  tmpn[l,n] = tmp[l,n]/sums[l]                      # ACT copy with per-partition scale
  attn[l,m] = sum_n vq[l,m,n]*tmpn[l,n]             # DVE bf16 2x mult + bf16 tree
  out = LN(attn + q)*gamma + beta                   # DVE stats, ACT rsqrt

Schedule notes (from NTFF trace analysis of the previous version):
  - small consts ride the scalar HWDGE queue so PE/ACT start at ~1.5us
  - vk stays f32 on the sync HWDGE queue (the A-step product is 1x on DVE
    regardless of dtype: the vexp broadcast is stride-0 on the innermost
    axis, which disables the 2x DVE mode, so casting vk buys nothing)
  - vq is cast f32->bf16 in flight on the gpsimd SWDGE queue; both bulk
    streams then run on different queues concurrently (one queue tops out
    around 420 GB/s read-side)
  - all vq SWDGE descriptors are generated up front so later gpsimd
    compute does not stall the vq stream
  - gpsimd takes one p-half product of the A-step for k-tiles 0-2 to pull
    DVE busy-time below the DMA window
"""

import numpy as np

B = 8
L = 512
KK = 512
MM = 64
NN = 64
PP = 64
DD = 64
NCORES = 8

LT = L // 128   # 4 l-tiles
KT = KK // 128  # 4 k-tiles
HALF = 32       # p-half / m-half within a tile

# which (k-tile, p-half) A-products run on gpsimd instead of DVE
GP_A = {(0, 1), (1, 1), (2, 1)}
# which (l-tile, m-half) C-products run on gpsimd instead of DVE
GP_C = set()

_CACHE = {}


def _patch_multiwait_split():
    """This environment's walrus accepts only ONE sem-wait per instruction,
    while Tile emits instructions carrying several. Rewrite the BIR JSON just
    before compilation: hoist excess waits onto single-wait NoOps inserted
    immediately before the offending instruction on the same engine."""
    import json

    from concourse import bass_utils, bass2jax

    if getattr(bass_utils, "_multiwait_split_patched", False):
        return

    orig = bass_utils.compile_bir_kernel

    def _split(bir_json):
        if isinstance(bir_json, bytes):
            m = json.loads(bir_json.decode())
        else:
            m = json.loads(bir_json)
        cnt = 0
        for fn in m["functions"]:
            for bb in fn["blocks"]:
                insts = bb["instructions"]
                out = []
                for inst in insts:
                    si = inst.get("sync_info")
                    waits = si.get("on_wait", []) if si else []
                    if len(waits) > 1:
                        for w in waits[:-1]:
                            cnt += 1
                            out.append(
                                {
                                    "name": f"WS-{cnt}-{inst['name']}",
                                    "opcode": "NoOp",
                                    "engine": inst["engine"],
                                    "ins": [],
                                    "outs": [],
                                    "debug": inst.get("debug", 0),
                                    "sync_info": {
                                        "on_update": [],
                                        "on_wait": [w],
                                    },
                                }
                            )
                        si["on_wait"] = [waits[-1]]
                    out.append(inst)
                bb["instructions"] = out
        return json.dumps(m).encode()

    def patched(bir_json, tmpdir, neff_name="file.neff", **kw):
        return orig(_split(bir_json), tmpdir, neff_name=neff_name, **kw)

    bass_utils.compile_bir_kernel = patched
    bass2jax.compile_bir_kernel = patched
    bass_utils._multiwait_split_patched = True


def _build_nc():
    import contextlib

    import concourse.bass as bass
    import concourse.tile as tile
    from concourse import mybir
    from concourse.masks import make_identity

    _patch_multiwait_split()

    f32 = mybir.dt.float32
    bf16 = mybir.dt.bfloat16
    Alu = mybir.AluOpType
    Act = mybir.ActivationFunctionType

    nc = bass.Bass()
    q_d = nc.dram_tensor("q", [L, DD], f32, kind="ExternalInput")
    k_d = nc.dram_tensor("k", [KK, DD], f32, kind="ExternalInput")
    vq_d = nc.dram_tensor("vq", [L, MM * NN], f32, kind="ExternalInput")
    vk_d = nc.dram_tensor("vk", [KK, PP * NN], f32, kind="ExternalInput")
    vexp_d = nc.dram_tensor("vexp", [KK, PP], f32, kind="ExternalInput")
    scale_d = nc.dram_tensor("scale", [1, 1], f32, kind="ExternalInput")
    gamma_d = nc.dram_tensor("ln_gamma", [1, DD], f32, kind="ExternalInput")
    beta_d = nc.dram_tensor("ln_beta", [1, DD], f32, kind="ExternalInput")
    out_d = nc.dram_tensor("out", [L, MM], f32, kind="ExternalOutput")

    with tile.TileContext(nc) as tc:
        lp_cm = nc.allow_low_precision("bf16 value-path partial sums")
        with lp_cm, contextlib.ExitStack() as ctx:
            const = ctx.enter_context(tc.tile_pool(name="const", bufs=1))
            vk_pool = ctx.enter_context(tc.tile_pool(name="vk", bufs=2 * KT))
            vq_pool = ctx.enter_context(tc.tile_pool(name="vq", bufs=2 * LT))
            prod_pool = ctx.enter_context(tc.tile_pool(name="prod", bufs=3))
            gprod_pool = ctx.enter_context(tc.tile_pool(name="gprod", bufs=2))
            tree_pool = ctx.enter_context(tc.tile_pool(name="tree", bufs=2))
            small = ctx.enter_context(tc.tile_pool(name="small", bufs=2))
            ps_scores = ctx.enter_context(
                tc.tile_pool(name="ps_s", bufs=2, space="PSUM")
            )
            ps_tmp_pool = ctx.enter_context(
                tc.tile_pool(name="ps_tmp", bufs=1, space="PSUM")
            )
            ps_tr = ctx.enter_context(tc.tile_pool(name="ps_tr", bufs=2, space="PSUM"))

            # ---- small consts on the scalar HWDGE queue (done ~1.5us) ----
            q_nat = const.tile([128, LT, DD], f32)
            nc.scalar.dma_start(
                out=q_nat, in_=q_d[:].rearrange("(t p) d -> p t d", p=128)
            )
            k_nat = const.tile([128, KT, DD], f32)
            nc.scalar.dma_start(
                out=k_nat, in_=k_d[:].rearrange("(t p) d -> p t d", p=128)
            )
            vexp_nat = const.tile([128, KT, PP], f32)
            nc.scalar.dma_start(
                out=vexp_nat, in_=vexp_d[:].rearrange("(t p) d -> p t d", p=128)
            )
            scale_bc = const.tile([128, 1], f32)
            nc.scalar.dma_start(out=scale_bc, in_=scale_d[:].to_broadcast([128, 1]))
            gamma_bc = const.tile([128, DD], f32)
            nc.scalar.dma_start(out=gamma_bc, in_=gamma_d[:].to_broadcast([128, DD]))
            beta_bc = const.tile([128, DD], f32)
            nc.scalar.dma_start(out=beta_bc, in_=beta_d[:].to_broadcast([128, DD]))

            # ---- vk f32 half-tiles on the sync HWDGE queue ----
            vk_halves = []
            for h in range(KT * 2):
                i, hh = divmod(h, 2)
                vkh = vk_pool.tile([128, HALF, NN], f32, tag="vk")
                nc.sync.dma_start(
                    out=vkh,
                    in_=vk_d[
                        i * 128 : (i + 1) * 128,
                        hh * HALF * NN : (hh + 1) * HALF * NN,
                    ],
                )
                vk_halves.append(vkh)

            # ---- vq bf16 half-tiles on the gpsimd SWDGE queue (cast in
            # flight); all descriptors generated before any gpsimd compute ----
            vq_halves = []
            for h in range(LT * 2):
                j, mh = divmod(h, 2)
                vqh = vq_pool.tile([128, HALF, NN], bf16, tag="vq")
                nc.gpsimd.dma_start(
                    out=vqh,
                    in_=vq_d[
                        j * 128 : (j + 1) * 128,
                        mh * HALF * NN : (mh + 1) * HALF * NN,
                    ],
                )
                vq_halves.append(vqh)

            # ---- constants ----
            identity = const.tile([128, 128], f32)
            make_identity(nc, identity)
            zero_t = const.tile([128, 1], f32)
            nc.vector.memset(zero_t, 0.0)
            eps_t = const.tile([128, 1], f32)
            nc.vector.memset(eps_t, 1e-3)

            # vkc carries a fused ones column (65th) so one matmul per
            # (k-tile, l-chunk) produces both tmp and the softmax denominator
            vkc = const.tile([128, KT, NN + 1], f32)
            nc.vector.memset(vkc[:, :, NN : NN + 1], 1.0)

            # ---- qT, kT via PE transpose ----
            qT = const.tile([64, L], f32)
            kT = const.tile([64, KK], f32)
            for i in range(LT):
                pq = ps_tr.tile([64, 128], f32, tag="tr")
                nc.tensor.transpose(pq, q_nat[:, i, :], identity)
                nc.scalar.copy(qT[:, i * 128 : (i + 1) * 128], pq)
            for i in range(KT):
                pk = ps_tr.tile([64, 128], f32, tag="tr")
                nc.tensor.transpose(pk, k_nat[:, i, :], identity)
                nc.scalar.copy(kT[:, i * 128 : (i + 1) * 128], pk)

            # ---- scoresT -> ET (all done by ~6us, PE+ACT) ----
            ET = const.tile([128, KT, L], f32)
            for i in range(KT):
                ps_s = ps_scores.tile([128, L], f32, tag="sc")
                nc.tensor.matmul(
                    ps_s,
                    lhsT=kT[:, i * 128 : (i + 1) * 128],
                    rhs=qT[:],
                    start=True,
                    stop=True,
                )
                nc.scalar.activation(
                    ET[:, i, :], ps_s, func=Act.Exp, bias=zero_t[:], scale=scale_bc[:]
                )

            # preload the Sqrt act table now (off the critical path); the
            # LN tail would otherwise eat a 1.3us table swap
            dummy = const.tile([128, 1], f32)
            nc.scalar.activation(dummy, eps_t, func=Act.Sqrt, bias=0.0, scale=1.0)

            # ---- A step + tmp/sums matmuls per k-tile ----
            ps_tmps = [
                ps_tmp_pool.tile(
                    [128, NN + 1], f32, tag=f"tmp{j}", name=f"ps_tmp{j}"
                )
                for j in range(LT)
            ]
            for i in range(KT):
                part = small.tile([128, 2, NN], bf16, tag="part")
                prs = []
                # products first (DVE half + possibly GP half run concurrently)
                for hh in range(2):
                    vkh = vk_halves[2 * i + hh]
                    on_gp = (i, hh) in GP_A
                    pool = gprod_pool if on_gp else prod_pool
                    eng = nc.gpsimd if on_gp else nc.vector
                    pr = pool.tile([128, HALF, NN], bf16, tag="pr")
                    eng.tensor_tensor(
                        pr[:],
                        vkh[:],
                        vexp_nat[
                            :, i, hh * HALF : (hh + 1) * HALF, None
                        ].to_broadcast([128, HALF, NN]),
                        Alu.mult,
                    )
                    prs.append(pr)
                # bf16 tree reduce over p (outer axis) on DVE, 2x mode
                for hh in range(2):
                    cur = prs[hh]
                    w = HALF // 2
                    while w >= 1:
                        if w == 1:
                            nxt = part[:, hh : hh + 1, :]
                        else:
                            nxt = tree_pool.tile([128, w, NN], bf16, tag=f"at{w}")
                        nc.vector.tensor_tensor(
                            nxt[:], cur[:, 0:w, :], cur[:, w : 2 * w, :], Alu.add
                        )
                        cur = nxt
                        w //= 2
                nc.vector.tensor_tensor(
                    vkc[:, i, 0:NN], part[:, 0, :], part[:, 1, :], Alu.add
                )
                # accumulate tmp|sums for every l-chunk: out has l on
                # partitions so no transposes are needed downstream
                for j in range(LT):
                    nc.tensor.matmul(
                        ps_tmps[j],
                        lhsT=ET[:, i, j * 128 : (j + 1) * 128],
                        rhs=vkc[:, i, :],
                        start=(i == 0),
                        stop=(i == KT - 1),
                    )

            # ---- fold 1/sums into tmp during the PSUM->SBUF copy ----
            tmp_sb = const.tile([128, LT, NN], bf16)
            for j in range(LT):
                recip = small.tile([128, 1], f32, tag="recip")
                nc.vector.reciprocal(recip, ps_tmps[j][:, NN : NN + 1])
                nc.scalar.activation(
                    tmp_sb[:, j, :],
                    ps_tmps[j][:, 0:NN],
                    func=Act.Copy,
                    bias=0.0,
                    scale=recip[:],
                )

            # ---- C step per l-tile: product+tree per m-half, then LN ----
            for j in range(LT):
                attn = small.tile([128, MM], f32, tag="attn")
                for mh in range(2):
                    vqh = vq_halves[2 * j + mh]
                    on_gp = (j, mh) in GP_C
                    pool = gprod_pool if on_gp else prod_pool
                    eng = nc.gpsimd if on_gp else nc.vector
                    pr2 = pool.tile([128, HALF, NN], bf16, tag="pr2")
                    eng.tensor_tensor(
                        pr2[:],
                        vqh[:],
                        tmp_sb[:, j, None, :].to_broadcast([128, HALF, NN]),
                        Alu.mult,
                    )
                    # bf16 tree over n (inner axis), 2x mode
                    cur = pr2
                    w = NN // 2
                    while w >= 1:
                        if w == 1:
                            nxt = attn[:, mh * HALF : (mh + 1) * HALF, None]
                        else:
                            nxt = tree_pool.tile([128, HALF, w], bf16, tag=f"ct{w}")
                        nc.vector.tensor_tensor(
                            nxt[:], cur[:, :, 0:w], cur[:, :, w : 2 * w], Alu.add
                        )
                        cur = nxt
                        w //= 2

                # x = attn + q ; LayerNorm(eps=1e-3)
                x = small.tile([128, MM], f32, tag="x")
                nc.vector.tensor_tensor(x, attn, q_nat[:, j, :], Alu.add)
                stats = small.tile([128, 6], f32, tag="stats")
                nc.vector.bn_stats(out=stats, in_=x[:])
                mv = small.tile([128, 2], f32, tag="mv")
                nc.vector.bn_aggr(out=mv, in_=stats[:])
                sd = small.tile([128, 1], f32, tag="sd")
                nc.scalar.activation(
                    sd, mv[:, 1:2], func=Act.Sqrt, bias=eps_t[:], scale=1.0
                )
                rstd = small.tile([128, 1], f32, tag="rstd")
                nc.vector.reciprocal(rstd, sd)
                xn = small.tile([128, MM], f32, tag="xn")
                nc.vector.tensor_scalar(
                    out=xn, in0=x, scalar1=mv[:, 0:1], scalar2=rstd,
                    op0=Alu.subtract, op1=Alu.mult,
                )
                xg = small.tile([128, MM], f32, tag="xg")
                nc.vector.tensor_tensor(xg, xn, gamma_bc, Alu.mult)
                out_t = small.tile([128, MM], f32, tag="out_t")
                nc.vector.tensor_tensor(out_t, xg, beta_bc, Alu.add)

                nc.scalar.dma_start(out=out_d[j * 128 : (j + 1) * 128, :], in_=out_t)

    return nc


def _get_nc():
    if "nc" not in _CACHE:
        _CACHE["nc"] = _build_nc()
    return _CACHE["nc"]


def kernel(q, k, vq, vk, vexp, scale, ln_gamma, ln_beta):
    from concourse import bass_utils

    nc = _get_nc()
    q = np.ascontiguousarray(np.asarray(q, dtype=np.float32))
    k = np.ascontiguousarray(np.asarray(k, dtype=np.float32))
    vq = np.ascontiguousarray(np.asarray(vq, dtype=np.float32)).reshape(B, L, MM * NN)
    vk = np.ascontiguousarray(np.asarray(vk, dtype=np.float32)).reshape(B, KK, PP * NN)
    vexp = np.ascontiguousarray(np.asarray(vexp, dtype=np.float32))
    scale_arr = np.asarray(scale, dtype=np.float32).reshape(1, 1)
    gamma_arr = np.asarray(ln_gamma, dtype=np.float32).reshape(1, DD)
    beta_arr = np.asarray(ln_beta, dtype=np.float32).reshape(1, DD)

    in_maps = [
        {
            "q": q[c],
            "k": k[c],
            "vq": vq[c],
            "vk": vk[c],
            "vexp": vexp[c],
            "scale": scale_arr,
            "ln_gamma": gamma_arr,
            "ln_beta": beta_arr,
        }
        for c in range(NCORES)
    ]
    res = bass_utils.run_bass_kernel_spmd(nc, in_maps, core_ids=list(range(NCORES)))
    out = np.stack([res.results[c]["out"] for c in range(NCORES)], axis=0)
    return out.astype(np.float32)


# revision 8
# speedup vs baseline: 1.0442x; 1.0442x over previous
"""Trainium2 Bass kernel for nn_CrossAttention (B=8, L=K=512, M=N=P=D=64).

One batch per NeuronCore (8 cores, data-parallel, no collectives).

Math per batch b:
  scoresT[k,l] = scale * (K @ Q^T)                  # PE, contract D=64
  ET = exp(scale*scoresT)                           # ACT (no max-sub: |s|<~45)
  vkc[k,n] = sum_p vk[k,p,n]*vexp[k,p]              # DVE+GPSIMD mult, DVE bf16 tree
  [tmp|sums][l,(n|1)] = ET_chunk^T @ [vkc|1]        # PE accumulate, l on partitions
  tmpn[l,n] = tmp[l,n]/sums[l]                      # ACT copy with per-partition scale
  attn[l,m] = sum_n vq[l,m,n]*tmpn[l,n]             # DVE bf16 2x mult + bf16 tree
  out = LN(attn + q)*gamma + beta                   # DVE stats, ACT sqrt

Layout: all row-indexed tensors use a (p,t) interleave — partition p of
tile t holds logical row 4p+t. Every DMA then moves >=1KB contiguous per
partition (line rate); q/k/vexp are one DMA each and the output is one
store. The interleave is self-consistent: softmax/contractions are
permutation-invariant along k, and the l permutation is identical for
q, vq, ET columns, tmp and the output store, which undoes it.

Schedule (from NTFF traces): total HBM per core caps ~430 GB/s shared
across queues, so vk (sync queue) gets strict priority and the vq SWDGE
issues are slotted mid-A-phase; gpsimd carries one p-half product per
early k-tile, with its tree deferred in the DVE program to avoid
head-of-line stalls.
"""

import numpy as np

B = 8
L = 512
KK = 512
MM = 64
NN = 64
PP = 64
DD = 64
NCORES = 8

LT = L // 128   # 4 l-tiles (slot index in the (p,t) interleave)
KT = KK // 128  # 4 k-tiles
HALF = 32       # p-half / m-half within a tile

_CACHE = {}


def _patch_multiwait_split():
    """This environment's walrus accepts only ONE sem-wait per instruction,
    while Tile emits instructions carrying several. Rewrite the BIR JSON just
    before compilation: hoist excess waits onto single-wait NoOps inserted
    immediately before the offending instruction on the same engine."""
    import json

    from concourse import bass_utils, bass2jax

    if getattr(bass_utils, "_multiwait_split_patched", False):
        return

    orig = bass_utils.compile_bir_kernel

    def _split(bir_json):
        if isinstance(bir_json, bytes):
            m = json.loads(bir_json.decode())
        else:
            m = json.loads(bir_json)
        cnt = 0
        for fn in m["functions"]:
            for bb in fn["blocks"]:
                insts = bb["instructions"]
                out = []
                for inst in insts:
                    si = inst.get("sync_info")
                    waits = si.get("on_wait", []) if si else []
                    if len(waits) > 1:
                        for w in waits[:-1]:
                            cnt += 1
                            out.append(
                                {
                                    "name": f"WS-{cnt}-{inst['name']}",
                                    "opcode": "NoOp",
                                    "engine": inst["engine"],
                                    "ins": [],
                                    "outs": [],
                                    "debug": inst.get("debug", 0),
                                    "sync_info": {
                                        "on_update": [],
                                        "on_wait": [w],
                                    },
                                }
                            )
                        si["on_wait"] = [waits[-1]]
                    out.append(inst)
                bb["instructions"] = out
        return json.dumps(m).encode()

    def patched(bir_json, tmpdir, neff_name="file.neff", **kw):
        return orig(_split(bir_json), tmpdir, neff_name=neff_name, **kw)

    bass_utils.compile_bir_kernel = patched
    bass2jax.compile_bir_kernel = patched
    bass_utils._multiwait_split_patched = True


def _build_nc():
    import contextlib

    import concourse.bass as bass
    import concourse.tile as tile
    from concourse import mybir
    from concourse.masks import make_identity

    _patch_multiwait_split()

    f32 = mybir.dt.float32
    bf16 = mybir.dt.bfloat16
    Alu = mybir.AluOpType
    Act = mybir.ActivationFunctionType

    nc = bass.Bass()
    q_d = nc.dram_tensor("q", [L, DD], f32, kind="ExternalInput")
    k_d = nc.dram_tensor("k", [KK, DD], f32, kind="ExternalInput")
    vq_d = nc.dram_tensor("vq", [L, MM * NN], f32, kind="ExternalInput")
    vk_d = nc.dram_tensor("vk", [KK, PP * NN], f32, kind="ExternalInput")
    vexp_d = nc.dram_tensor("vexp", [KK, PP], f32, kind="ExternalInput")
    scale_d = nc.dram_tensor("scale", [1, 1], f32, kind="ExternalInput")
    gamma_d = nc.dram_tensor("ln_gamma", [1, DD], f32, kind="ExternalInput")
    beta_d = nc.dram_tensor("ln_beta", [1, DD], f32, kind="ExternalInput")
    out_d = nc.dram_tensor("out", [L, MM], f32, kind="ExternalOutput")

    # (p,t)-interleaved DRAM views: partition p, slot t -> logical row 4p+t
    vk_v = vk_d[:].rearrange("(p t) c -> p t c", t=KT)
    vq_v = vq_d[:].rearrange("(p t) c -> p t c", t=LT)

    with tile.TileContext(nc) as tc:
        lp_cm = nc.allow_low_precision("bf16 value-path partial sums")
        with lp_cm, contextlib.ExitStack() as ctx:
            const = ctx.enter_context(tc.tile_pool(name="const", bufs=1))
            vk_pool = ctx.enter_context(tc.tile_pool(name="vk", bufs=2 * KT))
            vq_pool = ctx.enter_context(tc.tile_pool(name="vq", bufs=2 * LT))
            prod_pool = ctx.enter_context(tc.tile_pool(name="prod", bufs=3))
            gprod_pool = ctx.enter_context(tc.tile_pool(name="gprod", bufs=2))
            tree_pool = ctx.enter_context(tc.tile_pool(name="tree", bufs=2))
            small = ctx.enter_context(tc.tile_pool(name="small", bufs=2))
            ps_scores = ctx.enter_context(
                tc.tile_pool(name="ps_s", bufs=2, space="PSUM")
            )
            ps_tmp_pool = ctx.enter_context(
                tc.tile_pool(name="ps_tmp", bufs=1, space="PSUM")
            )
            ps_tr = ctx.enter_context(tc.tile_pool(name="ps_tr", bufs=2, space="PSUM"))

            # ---- small consts on the scalar HWDGE queue, line rate ----
            scale_bc = const.tile([128, 1], f32)
            nc.scalar.dma_start(out=scale_bc, in_=scale_d[:].to_broadcast([128, 1]))
            q_nat = const.tile([128, LT, DD], f32)
            nc.scalar.dma_start(
                out=q_nat, in_=q_d[:].rearrange("(p t) d -> p t d", t=LT)
            )
            k_nat = const.tile([128, KT, DD], f32)
            nc.scalar.dma_start(
                out=k_nat, in_=k_d[:].rearrange("(p t) d -> p t d", t=KT)
            )
            vexp_nat = const.tile([128, KT, PP], f32)
            nc.scalar.dma_start(
                out=vexp_nat, in_=vexp_d[:].rearrange("(p t) d -> p t d", t=KT)
            )
            gamma_bc = const.tile([128, DD], f32)
            nc.scalar.dma_start(out=gamma_bc, in_=gamma_d[:].to_broadcast([128, DD]))
            beta_bc = const.tile([128, DD], f32)
            nc.scalar.dma_start(out=beta_bc, in_=beta_d[:].to_broadcast([128, DD]))

            # ---- vk f32 half-tiles on the sync HWDGE queue (priority) ----
            vk_halves = []
            for h in range(KT * 2):
                i, hh = divmod(h, 2)
                vkh = vk_pool.tile([128, HALF, NN], f32, tag="vk", name=f"vkh{h}")
                nc.sync.dma_start(
                    out=vkh,
                    in_=vk_v[:, i, hh * HALF * NN : (hh + 1) * HALF * NN],
                )
                vk_halves.append(vkh)

            # vq half-tiles (bf16 cast, SWDGE): tiles allocated now, DMAs
            # issued later inside the gpsimd stream so vk keeps HBM priority
            vq_halves = [
                vq_pool.tile([128, HALF, NN], bf16, tag="vq", name=f"vqh{h}")
                for h in range(LT * 2)
            ]

            def issue_vq(h):
                j, mh = divmod(h, 2)
                nc.gpsimd.dma_start(
                    out=vq_halves[h],
                    in_=vq_v[:, j, mh * HALF * NN : (mh + 1) * HALF * NN],
                )

            # ---- constants ----
            identity = const.tile([128, 128], f32)
            make_identity(nc, identity)
            zero_t = const.tile([128, 1], f32)
            nc.vector.memset(zero_t, 0.0)
            eps_t = const.tile([128, 1], f32)
            nc.vector.memset(eps_t, 1e-3)

            # vkc carries a fused ones column (65th) so one matmul per
            # (k-tile, l-chunk) produces both tmp and the softmax denominator
            vkc = const.tile([128, KT, NN + 1], f32)
            nc.vector.memset(vkc[:, :, NN : NN + 1], 1.0)

            # ---- qT, kT via PE transpose ----
            qT = const.tile([64, L], f32)
            kT = const.tile([64, KK], f32)
            for i in range(LT):
                pq = ps_tr.tile([64, 128], f32, tag="tr")
                nc.tensor.transpose(pq, q_nat[:, i, :], identity)
                nc.scalar.copy(qT[:, i * 128 : (i + 1) * 128], pq)
            for i in range(KT):
                pk = ps_tr.tile([64, 128], f32, tag="tr")
                nc.tensor.transpose(pk, k_nat[:, i, :], identity)
                nc.scalar.copy(kT[:, i * 128 : (i + 1) * 128], pk)

            # ---- scoresT -> ET (all done by ~6us, PE+ACT) ----
            ET = const.tile([128, KT, L], f32)
            for i in range(KT):
                ps_s = ps_scores.tile([128, L], f32, tag="sc")
                nc.tensor.matmul(
                    ps_s,
                    lhsT=kT[:, i * 128 : (i + 1) * 128],
                    rhs=qT[:],
                    start=True,
                    stop=True,
                )
                nc.scalar.activation(
                    ET[:, i, :], ps_s, func=Act.Exp, bias=zero_t[:], scale=scale_bc[:]
                )

            # preload the Sqrt act table now (off the critical path); the
            # LN tail would otherwise eat a 1.3us table swap
            dummy = const.tile([128, 1], f32)
            nc.scalar.activation(dummy, eps_t, func=Act.Sqrt, bias=0.0, scale=1.0)

            # ---- A step + tmp/sums matmuls per k-tile ----
            # gpsimd owns the second p-half product of k-tiles 0-2; its tree
            # is deferred in the DVE stream until the next tile's own work so
            # the in-order DVE queue never blocks on the slower gpsimd.
            ps_tmps = [
                ps_tmp_pool.tile(
                    [128, NN + 1], f32, tag=f"tmp{j}", name=f"ps_tmp{j}"
                )
                for j in range(LT)
            ]
            parts = [
                small.tile([128, 2, NN], bf16, tag=f"part{i}", name=f"part{i}")
                for i in range(KT)
            ]

            def dve_tree_p(pr, out_slot):
                cur = pr
                w = HALF // 2
                while w >= 1:
                    if w == 1:
                        nxt = out_slot
                    else:
                        nxt = tree_pool.tile(
                            [128, w, NN], bf16, tag=f"at{w}", name=f"at{w}"
                        )
                    nc.vector.tensor_tensor(
                        nxt[:], cur[:, 0:w, :], cur[:, w : 2 * w, :], Alu.add
                    )
                    cur = nxt
                    w //= 2

            def a_product(eng, pool, i, hh):
                pr = pool.tile([128, HALF, NN], bf16, tag="pr", name=f"pr{i}{hh}")
                eng.tensor_tensor(
                    pr[:],
                    vk_halves[2 * i + hh][:],
                    vexp_nat[
                        :, i, hh * HALF : (hh + 1) * HALF, None
                    ].to_broadcast([128, HALF, NN]),
                    Alu.mult,
                )
                return pr

            def finish_tile(i):
                nc.vector.tensor_tensor(
                    vkc[:, i, 0:NN], parts[i][:, 0, :], parts[i][:, 1, :], Alu.add
                )
                for j in range(LT):
                    nc.tensor.matmul(
                        ps_tmps[j],
                        lhsT=ET[:, i, j * 128 : (j + 1) * 128],
                        rhs=vkc[:, i, :],
                        start=(i == 0),
                        stop=(i == KT - 1),
                    )

            GPA = (0, 1, 2)  # k-tiles whose h1 product runs on gpsimd
            gp_pr = {}
            deferred = []
            for i in range(KT):
                prD = a_product(nc.vector, prod_pool, i, 0)
                if i in GPA:
                    gp_pr[i] = a_product(nc.gpsimd, gprod_pool, i, 1)
                    if i == 1:
                        # vq stream starts once vk is ~70% through the queue
                        for h in range(2 * LT):
                            issue_vq(h)
                dve_tree_p(prD, parts[i][:, 0:1, :])
                if i not in GPA:
                    prD2 = a_product(nc.vector, prod_pool, i, 1)
                    dve_tree_p(prD2, parts[i][:, 1:2, :])
                # process previous tile's gpsimd half now (it has finished
                # during this tile's DVE work)
                if deferred:
                    pi = deferred.pop()
                    dve_tree_p(gp_pr[pi], parts[pi][:, 1:2, :])
                    finish_tile(pi)
                if i in GPA:
                    deferred.append(i)
                else:
                    finish_tile(i)
            while deferred:
                pi = deferred.pop()
                dve_tree_p(gp_pr[pi], parts[pi][:, 1:2, :])
                finish_tile(pi)

            # ---- fold 1/sums into tmp during the PSUM->SBUF copy ----
            tmp_sb = const.tile([128, LT, NN], bf16)
            for j in range(LT):
                recip = small.tile([128, 1], f32, tag="recip")
                nc.vector.reciprocal(recip, ps_tmps[j][:, NN : NN + 1])
                nc.scalar.activation(
                    tmp_sb[:, j, :],
                    ps_tmps[j][:, 0:NN],
                    func=Act.Copy,
                    bias=0.0,
                    scale=recip[:],
                )

            # ---- C step per l-tile: product+tree per m-half, then LN ----
            out_sb = const.tile([128, LT, MM], f32)

            def c_product(eng, pool, j, mh):
                pr2 = pool.tile([128, HALF, NN], bf16, tag="pr2", name=f"pr2{j}{mh}")
                eng.tensor_tensor(
                    pr2[:],
                    vq_halves[2 * j + mh][:],
                    tmp_sb[:, j, None, :].to_broadcast([128, HALF, NN]),
                    Alu.mult,
                )
                return pr2

            def dve_tree_n(pr2, attn, mh):
                cur = pr2
                w = NN // 2
                while w >= 1:
                    if w == 1:
                        nxt = attn[:, mh * HALF : (mh + 1) * HALF, None]
                    else:
                        nxt = tree_pool.tile(
                            [128, HALF, w], bf16, tag=f"ct{w}", name=f"ct{w}"
                        )
                    nc.vector.tensor_tensor(
                        nxt[:], cur[:, :, 0:w], cur[:, :, w : 2 * w], Alu.add
                    )
                    cur = nxt
                    w //= 2

            def layer_norm(j, attn):
                x = small.tile([128, MM], f32, tag="x")
                nc.vector.tensor_tensor(x, attn, q_nat[:, j, :], Alu.add)
                stats = small.tile([128, 6], f32, tag="stats")
                nc.vector.bn_stats(out=stats, in_=x[:])
                mv = small.tile([128, 2], f32, tag="mv")
                nc.vector.bn_aggr(out=mv, in_=stats[:])
                sd = small.tile([128, 1], f32, tag="sd")
                nc.scalar.activation(
                    sd, mv[:, 1:2], func=Act.Sqrt, bias=eps_t[:], scale=1.0
                )
                rstd = small.tile([128, 1], f32, tag="rstd")
                nc.vector.reciprocal(rstd, sd)
                xn = small.tile([128, MM], f32, tag="xn")
                nc.vector.tensor_scalar(
                    out=xn, in0=x, scalar1=mv[:, 0:1], scalar2=rstd,
                    op0=Alu.subtract, op1=Alu.mult,
                )
                xg = small.tile([128, MM], f32, tag="xg")
                nc.vector.tensor_tensor(xg, xn, gamma_bc, Alu.mult)
                nc.vector.tensor_tensor(out_sb[:, j, :], xg, beta_bc, Alu.add)

            # gpsimd takes tile 0's first m-half; its tree is deferred past
            # tile 1's DVE work
            attns = [
                small.tile([128, MM], f32, tag=f"attn{j}", name=f"attn{j}")
                for j in range(LT)
            ]
            g2 = c_product(nc.gpsimd, gprod_pool, 0, 0)
            dve_tree_n(c_product(nc.vector, prod_pool, 0, 1), attns[0], 1)
            for mh in range(2):
                dve_tree_n(c_product(nc.vector, prod_pool, 1, mh), attns[1], mh)
            dve_tree_n(g2, attns[0], 0)
            layer_norm(0, attns[0])
            layer_norm(1, attns[1])
            for j in (2, 3):
                for mh in range(2):
                    dve_tree_n(c_product(nc.vector, prod_pool, j, mh), attns[j], mh)
                layer_norm(j, attns[j])

            # single line-rate store of the whole output
            nc.scalar.dma_start(
                out=out_d[:].rearrange("(p t) d -> p t d", t=LT), in_=out_sb
            )

    return nc


def _get_nc():
    if "nc" not in _CACHE:
        _CACHE["nc"] = _build_nc()
    return _CACHE["nc"]


def kernel(q, k, vq, vk, vexp, scale, ln_gamma, ln_beta):
    from concourse import bass_utils

    nc = _get_nc()
    q = np.ascontiguousarray(np.asarray(q, dtype=np.float32))
    k = np.ascontiguousarray(np.asarray(k, dtype=np.float32))
    vq = np.ascontiguousarray(np.asarray(vq, dtype=np.float32)).reshape(B, L, MM * NN)
    vk = np.ascontiguousarray(np.asarray(vk, dtype=np.float32)).reshape(B, KK, PP * NN)
    vexp = np.ascontiguousarray(np.asarray(vexp, dtype=np.float32))
    scale_arr = np.asarray(scale, dtype=np.float32).reshape(1, 1)
    gamma_arr = np.asarray(ln_gamma, dtype=np.float32).reshape(1, DD)
    beta_arr = np.asarray(ln_beta, dtype=np.float32).reshape(1, DD)

    in_maps = [
        {
            "q": q[c],
            "k": k[c],
            "vq": vq[c],
            "vk": vk[c],
            "vexp": vexp[c],
            "scale": scale_arr,
            "ln_gamma": gamma_arr,
            "ln_beta": beta_arr,
        }
        for c in range(NCORES)
    ]
    res = bass_utils.run_bass_kernel_spmd(nc, in_maps, core_ids=list(range(NCORES)))
    out = np.stack([res.results[c]["out"] for c in range(NCORES)], axis=0)
    return out.astype(np.float32)


# revision 10
# speedup vs baseline: 1.2267x; 1.1748x over previous
"""Trainium2 Bass kernel for nn_CrossAttention (B=8, L=K=512, M=N=P=D=64).

One batch per NeuronCore (8 cores, data-parallel, no collectives).

Math per batch b:
  scoresT[k,l] = scale * (K @ Q^T)                  # PE, contract D=64
  ET = exp(scale*scoresT)                           # ACT (no max-sub: |s|<~45)
  VE[k,p,n] = vexp[k,p] broadcast over n            # ACT copy (cast bf16)
  vkc[k,n] = sum_p vk[k,p,n]*VE[k,p,n]              # DVE bf16 2x mult + tree
  [tmp|sums][l,(n|1)] = ET_chunk^T @ [vkc|1]        # PE accumulate, l on partitions
  tmpn[l,n] = tmp[l,n]/sums[l]                      # ACT copy with per-partition scale
  attn[l,m] = sum_n vq[l,m,n]*tmpn[l,n]             # DVE bf16 2x mult + tree
  out = LN(attn + q)*gamma + beta                   # DVE stats, ACT sqrt

Layout: all row-indexed tensors use a (p,t) interleave — partition p of
tile t holds logical row 4p+t — so every DMA moves >=1KB contiguous per
partition (line rate). The interleave is self-consistent along both k
and l and the single output store undoes it.

Schedule (from NTFF traces of previous versions):
  - per-core HBM caps ~430 GB/s TOTAL across queues, so all 16MB of bulk
    (vk then vq, f32->bf16 cast in flight) rides ONE SWDGE queue whose
    FIFO gives vk strict priority; q/k/vexp ride the sync queue (done
    ~1.5us); only the slow to_broadcast splats (gamma/beta) and the
    final store use the scalar queue.
  - gpsimd runs NO compute: DVE ops overlapping gpsimd tensor ops were
    measured 4-8x slower (SBUF contention).
  - the A-step product runs at DVE 2x by materializing the vexp
    broadcast into a real bf16 tile on the otherwise-idle ACT engine
    (a stride-0 innermost operand would force 1x on the DVE).
  - scale is broadcast on-chip via a ones-row PE matmul instead of a
    128-descriptor splat DMA (it gates the exps).
"""

import numpy as np

B = 8
L = 512
KK = 512
MM = 64
NN = 64
PP = 64
DD = 64
NCORES = 8

LT = L // 128   # 4 l-tiles (slot index in the (p,t) interleave)
KT = KK // 128  # 4 k-tiles
HALF = 32       # p-half / m-half within a tile

_CACHE = {}


def _patch_multiwait_split():
    """This environment's walrus accepts only ONE sem-wait per instruction,
    while Tile emits instructions carrying several. Rewrite the BIR JSON just
    before compilation: hoist excess waits onto single-wait NoOps inserted
    immediately before the offending instruction on the same engine."""
    import json

    from concourse import bass_utils, bass2jax

    if getattr(bass_utils, "_multiwait_split_patched", False):
        return

    orig = bass_utils.compile_bir_kernel

    def _split(bir_json):
        if isinstance(bir_json, bytes):
            m = json.loads(bir_json.decode())
        else:
            m = json.loads(bir_json)
        cnt = 0
        for fn in m["functions"]:
            for bb in fn["blocks"]:
                insts = bb["instructions"]
                out = []
                for inst in insts:
                    si = inst.get("sync_info")
                    waits = si.get("on_wait", []) if si else []
                    if len(waits) > 1:
                        for w in waits[:-1]:
                            cnt += 1
                            out.append(
                                {
                                    "name": f"WS-{cnt}-{inst['name']}",
                                    "opcode": "NoOp",
                                    "engine": inst["engine"],
                                    "ins": [],
                                    "outs": [],
                                    "debug": inst.get("debug", 0),
                                    "sync_info": {
                                        "on_update": [],
                                        "on_wait": [w],
                                    },
                                }
                            )
                        si["on_wait"] = [waits[-1]]
                    out.append(inst)
                bb["instructions"] = out
        return json.dumps(m).encode()

    def patched(bir_json, tmpdir, neff_name="file.neff", **kw):
        return orig(_split(bir_json), tmpdir, neff_name=neff_name, **kw)

    bass_utils.compile_bir_kernel = patched
    bass2jax.compile_bir_kernel = patched
    bass_utils._multiwait_split_patched = True


def _build_nc():
    import contextlib

    import concourse.bass as bass
    import concourse.tile as tile
    from concourse import mybir
    from concourse.masks import make_identity

    _patch_multiwait_split()

    f32 = mybir.dt.float32
    bf16 = mybir.dt.bfloat16
    Alu = mybir.AluOpType
    Act = mybir.ActivationFunctionType

    nc = bass.Bass()
    q_d = nc.dram_tensor("q", [L, DD], f32, kind="ExternalInput")
    k_d = nc.dram_tensor("k", [KK, DD], f32, kind="ExternalInput")
    vq_d = nc.dram_tensor("vq", [L, MM * NN], f32, kind="ExternalInput")
    vk_d = nc.dram_tensor("vk", [KK, PP * NN], f32, kind="ExternalInput")
    vexp_d = nc.dram_tensor("vexp", [KK, PP], f32, kind="ExternalInput")
    scale_d = nc.dram_tensor("scale", [1, 1], f32, kind="ExternalInput")
    gamma_d = nc.dram_tensor("ln_gamma", [1, DD], f32, kind="ExternalInput")
    beta_d = nc.dram_tensor("ln_beta", [1, DD], f32, kind="ExternalInput")
    out_d = nc.dram_tensor("out", [L, MM], f32, kind="ExternalOutput")

    # (p,t)-interleaved DRAM views: partition p, slot t -> logical row 4p+t
    vk_v = vk_d[:].rearrange("(p t) c -> p t c", t=KT)
    vq_v = vq_d[:].rearrange("(p t) c -> p t c", t=LT)

    with tile.TileContext(nc) as tc:
        lp_cm = nc.allow_low_precision("bf16 value-path partial sums")
        with lp_cm, contextlib.ExitStack() as ctx:
            const = ctx.enter_context(tc.tile_pool(name="const", bufs=1))
            vk_pool = ctx.enter_context(tc.tile_pool(name="vk", bufs=2 * KT))
            vq_pool = ctx.enter_context(tc.tile_pool(name="vq", bufs=2 * LT))
            ve_pool = ctx.enter_context(tc.tile_pool(name="ve", bufs=3))
            prod_pool = ctx.enter_context(tc.tile_pool(name="prod", bufs=3))
            tree_pool = ctx.enter_context(tc.tile_pool(name="tree", bufs=2))
            small = ctx.enter_context(tc.tile_pool(name="small", bufs=2))
            ps_scores = ctx.enter_context(
                tc.tile_pool(name="ps_s", bufs=2, space="PSUM")
            )
            ps_tmp_pool = ctx.enter_context(
                tc.tile_pool(name="ps_tmp", bufs=1, space="PSUM")
            )
            ps_tr = ctx.enter_context(tc.tile_pool(name="ps_tr", bufs=1, space="PSUM"))

            # ---- small line-rate loads on the sync HWDGE queue ----
            q_nat = const.tile([128, LT, DD], f32)
            nc.sync.dma_start(
                out=q_nat, in_=q_d[:].rearrange("(p t) d -> p t d", t=LT)
            )
            k_nat = const.tile([128, KT, DD], f32)
            nc.sync.dma_start(
                out=k_nat, in_=k_d[:].rearrange("(p t) d -> p t d", t=KT)
            )
            vexp_nat = const.tile([128, KT, PP], f32)
            nc.sync.dma_start(
                out=vexp_nat, in_=vexp_d[:].rearrange("(p t) d -> p t d", t=KT)
            )
            scale_sb = const.tile([1, 1], f32)
            nc.sync.dma_start(out=scale_sb, in_=scale_d[:])

            # slow splat DMAs isolated on the scalar queue (needed late)
            gamma_bc = const.tile([128, DD], f32)
            nc.scalar.dma_start(out=gamma_bc, in_=gamma_d[:].to_broadcast([128, DD]))
            beta_bc = const.tile([128, DD], f32)
            nc.scalar.dma_start(out=beta_bc, in_=beta_d[:].to_broadcast([128, DD]))

            # ---- bulk: ONE SWDGE queue, bf16 cast in flight, vk first ----
            vk_halves = []
            for h in range(KT * 2):
                i, hh = divmod(h, 2)
                vkh = vk_pool.tile([128, HALF, NN], bf16, tag="vk", name=f"vkh{h}")
                nc.gpsimd.dma_start(
                    out=vkh,
                    in_=vk_v[:, i, hh * HALF * NN : (hh + 1) * HALF * NN],
                )
                vk_halves.append(vkh)
            vq_halves = []
            for h in range(LT * 2):
                j, mh = divmod(h, 2)
                vqh = vq_pool.tile([128, HALF, NN], bf16, tag="vq", name=f"vqh{h}")
                nc.gpsimd.dma_start(
                    out=vqh,
                    in_=vq_v[:, j, mh * HALF * NN : (mh + 1) * HALF * NN],
                )
                vq_halves.append(vqh)

            # ---- constants ----
            identity = const.tile([128, 128], f32)
            make_identity(nc, identity)
            zero_t = const.tile([128, 1], f32)
            nc.vector.memset(zero_t, 0.0)
            eps_t = const.tile([128, 1], f32)
            nc.vector.memset(eps_t, 1e-3)
            ones_row = const.tile([1, 128], f32)
            nc.vector.memset(ones_row, 1.0)

            # scale broadcast on-chip: [1,1] -> [128,1] via ones-row matmul
            ps_bc = ps_tr.tile([128, 1], f32, tag="bc")
            nc.tensor.matmul(ps_bc, lhsT=ones_row, rhs=scale_sb, start=True, stop=True)
            scale_bc = const.tile([128, 1], f32)
            nc.scalar.copy(scale_bc, ps_bc)

            # vkc carries a fused ones column (65th) so one matmul per
            # (k-tile, l-chunk) produces both tmp and the softmax denominator
            vkc = const.tile([128, KT, NN + 1], f32)
            nc.vector.memset(vkc[:, :, NN : NN + 1], 1.0)

            # ---- qT, kT via PE transpose ----
            qT = const.tile([64, L], f32)
            kT = const.tile([64, KK], f32)
            for i in range(LT):
                pq = ps_tr.tile([64, 128], f32, tag="tr")
                nc.tensor.transpose(pq, q_nat[:, i, :], identity)
                nc.scalar.copy(qT[:, i * 128 : (i + 1) * 128], pq)
            for i in range(KT):
                pk = ps_tr.tile([64, 128], f32, tag="tr")
                nc.tensor.transpose(pk, k_nat[:, i, :], identity)
                nc.scalar.copy(kT[:, i * 128 : (i + 1) * 128], pk)

            # ---- scoresT -> ET (all done by ~6us, PE+ACT) ----
            ET = const.tile([128, KT, L], f32)
            for i in range(KT):
                ps_s = ps_scores.tile([128, L], f32, tag="sc")
                nc.tensor.matmul(
                    ps_s,
                    lhsT=kT[:, i * 128 : (i + 1) * 128],
                    rhs=qT[:],
                    start=True,
                    stop=True,
                )
                nc.scalar.activation(
                    ET[:, i, :], ps_s, func=Act.Exp, bias=zero_t[:], scale=scale_bc[:]
                )

            # preload the Sqrt act table now (off the critical path); the
            # LN tail would otherwise eat a 1.3us table swap
            dummy = const.tile([128, 1], f32)
            nc.scalar.activation(dummy, eps_t, func=Act.Sqrt, bias=0.0, scale=1.0)

            # ---- A step + tmp/sums matmuls per k-tile ----
            ps_tmps = [
                ps_tmp_pool.tile(
                    [128, NN + 1], f32, tag=f"tmp{j}", name=f"ps_tmp{j}"
                )
                for j in range(LT)
            ]

            def dve_tree_p(pr, out_slot):
                cur = pr
                w = HALF // 2
                while w >= 1:
                    if w == 1:
                        nxt = out_slot
                    else:
                        nxt = tree_pool.tile(
                            [128, w, NN], bf16, tag=f"at{w}", name=f"at{w}"
                        )
                    nc.vector.tensor_tensor(
                        nxt[:], cur[:, 0:w, :], cur[:, w : 2 * w, :], Alu.add
                    )
                    cur = nxt
                    w //= 2

            for i in range(KT):
                part = small.tile([128, 2, NN], bf16, tag="part")
                for hh in range(2):
                    # materialize the vexp broadcast as a real bf16 tile on
                    # ACT so the DVE product gets 2x (packed operands only)
                    ve = ve_pool.tile([128, HALF, NN], bf16, tag="ve", name=f"ve{i}{hh}")
                    nc.scalar.activation(
                        ve[:],
                        vexp_nat[
                            :, i, hh * HALF : (hh + 1) * HALF, None
                        ].to_broadcast([128, HALF, NN]),
                        func=Act.Copy,
                        bias=0.0,
                        scale=1.0,
                    )
                    pr = prod_pool.tile(
                        [128, HALF, NN], bf16, tag="pr", name=f"pr{i}{hh}"
                    )
                    nc.vector.tensor_tensor(
                        pr[:], vk_halves[2 * i + hh][:], ve[:], Alu.mult
                    )
                    dve_tree_p(pr, part[:, hh : hh + 1, :])
                nc.vector.tensor_tensor(
                    vkc[:, i, 0:NN], part[:, 0, :], part[:, 1, :], Alu.add
                )
                for j in range(LT):
                    nc.tensor.matmul(
                        ps_tmps[j],
                        lhsT=ET[:, i, j * 128 : (j + 1) * 128],
                        rhs=vkc[:, i, :],
                        start=(i == 0),
                        stop=(i == KT - 1),
                    )

            # ---- fold 1/sums into tmp during the PSUM->SBUF copy ----
            tmp_sb = const.tile([128, LT, NN], bf16)
            for j in range(LT):
                recip = small.tile([128, 1], f32, tag="recip")
                nc.vector.reciprocal(recip, ps_tmps[j][:, NN : NN + 1])
                nc.scalar.activation(
                    tmp_sb[:, j, :],
                    ps_tmps[j][:, 0:NN],
                    func=Act.Copy,
                    bias=0.0,
                    scale=recip[:],
                )

            # ---- C step per l-tile: product+tree per m-half, then LN ----
            out_sb = const.tile([128, LT, MM], f32)
            for j in range(LT):
                attn = small.tile([128, MM], f32, tag="attn")
                for mh in range(2):
                    pr2 = prod_pool.tile(
                        [128, HALF, NN], bf16, tag="pr2", name=f"pr2{j}{mh}"
                    )
                    nc.vector.tensor_tensor(
                        pr2[:],
                        vq_halves[2 * j + mh][:],
                        tmp_sb[:, j, None, :].to_broadcast([128, HALF, NN]),
                        Alu.mult,
                    )
                    cur = pr2
                    w = NN // 2
                    while w >= 1:
                        if w == 1:
                            nxt = attn[:, mh * HALF : (mh + 1) * HALF, None]
                        else:
                            nxt = tree_pool.tile(
                                [128, HALF, w], bf16, tag=f"ct{w}", name=f"ct{w}"
                            )
                        nc.vector.tensor_tensor(
                            nxt[:], cur[:, :, 0:w], cur[:, :, w : 2 * w], Alu.add
                        )
                        cur = nxt
                        w //= 2

                x = small.tile([128, MM], f32, tag="x")
                nc.vector.tensor_tensor(x, attn, q_nat[:, j, :], Alu.add)
                stats = small.tile([128, 6], f32, tag="stats")
                nc.vector.bn_stats(out=stats, in_=x[:])
                mv = small.tile([128, 2], f32, tag="mv")
                nc.vector.bn_aggr(out=mv, in_=stats[:])
                sd = small.tile([128, 1], f32, tag="sd")
                nc.scalar.activation(
                    sd, mv[:, 1:2], func=Act.Sqrt, bias=eps_t[:], scale=1.0
                )
                rstd = small.tile([128, 1], f32, tag="rstd")
                nc.vector.reciprocal(rstd, sd)
                xn = small.tile([128, MM], f32, tag="xn")
                nc.vector.tensor_scalar(
                    out=xn, in0=x, scalar1=mv[:, 0:1], scalar2=rstd,
                    op0=Alu.subtract, op1=Alu.mult,
                )
                xg = small.tile([128, MM], f32, tag="xg")
                nc.vector.tensor_tensor(xg, xn, gamma_bc, Alu.mult)
                nc.vector.tensor_tensor(out_sb[:, j, :], xg, beta_bc, Alu.add)

            # single line-rate store of the whole output
            nc.scalar.dma_start(
                out=out_d[:].rearrange("(p t) d -> p t d", t=LT), in_=out_sb
            )

    return nc


def _get_nc():
    if "nc" not in _CACHE:
        _CACHE["nc"] = _build_nc()
    return _CACHE["nc"]


def kernel(q, k, vq, vk, vexp, scale, ln_gamma, ln_beta):
    from concourse import bass_utils

    nc = _get_nc()
    q = np.ascontiguousarray(np.asarray(q, dtype=np.float32))
    k = np.ascontiguousarray(np.asarray(k, dtype=np.float32))
    vq = np.ascontiguousarray(np.asarray(vq, dtype=np.float32)).reshape(B, L, MM * NN)
    vk = np.ascontiguousarray(np.asarray(vk, dtype=np.float32)).reshape(B, KK, PP * NN)
    vexp = np.ascontiguousarray(np.asarray(vexp, dtype=np.float32))
    scale_arr = np.asarray(scale, dtype=np.float32).reshape(1, 1)
    gamma_arr = np.asarray(ln_gamma, dtype=np.float32).reshape(1, DD)
    beta_arr = np.asarray(ln_beta, dtype=np.float32).reshape(1, DD)

    in_maps = [
        {
            "q": q[c],
            "k": k[c],
            "vq": vq[c],
            "vk": vk[c],
            "vexp": vexp[c],
            "scale": scale_arr,
            "ln_gamma": gamma_arr,
            "ln_beta": beta_arr,
        }
        for c in range(NCORES)
    ]
    res = bass_utils.run_bass_kernel_spmd(nc, in_maps, core_ids=list(range(NCORES)))
    out = np.stack([res.results[c]["out"] for c in range(NCORES)], axis=0)
    return out.astype(np.float32)
